# revision 19
# baseline (speedup 1.0000x reference)
"""Cross-modal attention kernel for Trainium2 -- data-parallel over batch on 8 cores.

Reference computation per sample (C=256, H=W=64, N=H*W=4096, dqk=32):
    q = Wq @ x + bq; k = Wk @ y + bk; v = Wv @ y + bv
    out = gamma * (v @ softmax_j(q^T k)^T) + x

Strategy (per core = one batch sample):
  - Projections run in float32r, attention in bf16/fp8 so PE matmuls stream
    at 1 cycle/row (fp32 would be 4).
  - Energy is computed TRANSPOSED (E^T[j,i], keys on partitions) so the
    attention-weighted sum contracts over the partition dim with no
    transposes.  exp() is applied unnormalized (logits are O(1) by
    construction: gain-0.02 weights), softmax normalization happens on the
    [C, IBLK] output instead of the [N, N] matrix.
  - The K=32 energy matmuls are 4-way row-packed (tile_position).
  - exp(E^T) and v^T are stored fp8e4m3; AV and the denominator both run as
    MatmulPerfMode.DoubleRow contractions (2 fp8 weights/PE cell), pairing
    consecutive j-tiles via 3D [K,2,N] APs.  The denominator is a DoubleRow
    ones-matmul accumulating sum_j exp(E^T)[j,i] in PSUM.
  - Software pipelining: AV for group g-2 issues after the energy matmuls of
    group g; block n's normalization tail is deferred into block n+1.

Differences from the bf16 version:
  - exp(E^T) and v^T are stored as fp8e4m3; the AV contraction runs in
    MatmulPerfMode.DoubleRow (2 fp8 weights per PE cell -> half the cycles),
    pairing consecutive j-tiles along the partition dim via 3D [K,2,N] APs.
  - The softmax denominator is ALSO a DoubleRow matmul: ones[128,2,128] as
    stationary -> den[i] accumulates sum_j exp(E^T)[j,i] in PSUM, which
    removes the whole DVE accumulate+fold chain of the bf16 version.
  - gamma is applied as a per-partition tensor_scalar multiply on 1/den.
"""

import sys

if "/opt/trn_rl_repo" not in sys.path:
    sys.path.insert(0, "/opt/trn_rl_repo")

import numpy as np

import concourse.bacc as bacc
import concourse.mybir as mybir
import concourse.tile as tile
from concourse.bass_utils import run_bass_kernel_spmd

F32 = mybir.dt.float32
F32R = mybir.dt.float32r
BF16 = mybir.dt.bfloat16
FP8 = mybir.dt.float8e4

B, C, HW, D = 8, 256, 4096, 32
CH = C // 128
IBLK = 512
NIB = HW // IBLK
NJT = HW // 128
NPAIR = NJT // 2
EXPF = mybir.ActivationFunctionType.Exp
MULT = mybir.AluOpType.mult
ADD = mybir.AluOpType.add
DROW = mybir.MatmulPerfMode.DoubleRow


def _build():
    nc = bacc.Bacc("TRN2", target_bir_lowering=False, debug=False, num_devices=8)

    xr = nc.dram_tensor("xr", [C, HW], F32R, kind="ExternalInput")
    xf = nc.dram_tensor("xf", [C, HW], F32, kind="ExternalInput")
    yr = nc.dram_tensor("yr", [C, HW], F32R, kind="ExternalInput")
    wqT = nc.dram_tensor("wqT", [C, D], F32R, kind="ExternalInput")
    wkT = nc.dram_tensor("wkT", [C, D], F32R, kind="ExternalInput")
    wvT = nc.dram_tensor("wvT", [C, C], F32R, kind="ExternalInput")
    bqd = nc.dram_tensor("bqd", [D, 1], F32, kind="ExternalInput")
    bkd = nc.dram_tensor("bkd", [D, 1], F32, kind="ExternalInput")
    gbvd = nc.dram_tensor("gbvd", [128, CH], F32, kind="ExternalInput")
    gmd = nc.dram_tensor("gmd", [128, 1], F32, kind="ExternalInput")
    out = nc.dram_tensor("out", [C, HW], F32, kind="ExternalOutput")

    tc = tile.TileContext(nc)
    with tc:
        with (
            tc.tile_pool(name="cst", bufs=1) as cst,
            tc.tile_pool(name="qkv", bufs=1) as qkv,
        ):
            wq_sb = cst.tile([128, CH * D], F32R)
            wk_sb = cst.tile([128, CH * D], F32R)
            wv_sb = cst.tile([128, CH * C], F32R)
            bq_sb = cst.tile([D, 1], F32)
            bk_sb = cst.tile([D, 1], F32)
            gbv_sb = cst.tile([128, CH], F32)
            gm_sb = cst.tile([128, 1], F32)
            ones_sb = cst.tile([128, 2 * 128], FP8)
            nc.vector.memset(ones_sb[:], 1.0)
            nc.gpsimd.dma_start(bq_sb[:], bqd[:])
            nc.gpsimd.dma_start(bk_sb[:], bkd[:])
            nc.gpsimd.dma_start(gbv_sb[:], gbvd[:])
            nc.gpsimd.dma_start(gm_sb[:], gmd[:])

            q4 = qkv.tile([128, HW], BF16)
            k4 = qkv.tile([128, HW], BF16)
            vt = qkv.tile([128, NJT * C], FP8)

            NG = NJT // 4
            ptp = None  # assigned when the phase-B pools open
            psE = None

            def et_group(n, g, pt):
                # energy for (i-block n, group g): 4 row-packed K=32 matmuls
                # into two 2-bank psum tiles, then exp into pt (fp8)
                ets = [
                    psE.tile([128, 2 * IBLK], F32,
                             name=f"et{h}_{n}_{g}", tag="et", bufs=2)
                    for h in range(2)
                ]
                for q in range(4):
                    jt = 4 * g + q
                    nc.tensor.matmul(
                        ets[q // 2][:, (q % 2) * IBLK:(q % 2 + 1) * IBLK],
                        k4[32 * q:32 * (q + 1), jt * 128:(jt + 1) * 128],
                        q4[32 * q:32 * (q + 1), n * IBLK:(n + 1) * IBLK],
                        start=True,
                        stop=True,
                        tile_position=(32 * q, 0),
                    )
                for h in range(2):
                    nc.scalar.activation(
                        pt[:, (4 * g + 2 * h) * IBLK:(4 * g + 2 * h + 2) * IBLK],
                        ets[h][:], EXPF,
                    )

            with (
                tc.tile_pool(name="xy", bufs=1) as xy,
                tc.tile_pool(name="psA", bufs=4, space="PSUM") as psA,
            ):
                xr_sb = xy.tile([128, CH * HW], F32R)
                yr_sb = xy.tile([128, CH * HW], F32R)

                def in_chunk(src, dst_sb, h, c0, c1):
                    nc.sync.dma_start(
                        dst_sb[:, h * HW + c0: h * HW + c1],
                        src[h * 128:(h + 1) * 128, c0:c1],
                    )

                for h in range(CH):
                    nc.sync.dma_start(wq_sb[:, h * D:(h + 1) * D], wqT[h * 128:(h + 1) * 128, :])
                for h in range(CH):
                    in_chunk(xr, xr_sb, h, 0, IBLK)
                for h in range(CH):
                    nc.sync.dma_start(wk_sb[:, h * D:(h + 1) * D], wkT[h * 128:(h + 1) * 128, :])
                for h in range(CH):
                    in_chunk(yr, yr_sb, h, 0, IBLK)
                for h in range(CH):
                    nc.sync.dma_start(wv_sb[:, h * C:(h + 1) * C], wvT[h * 128:(h + 1) * 128, :])
                for ic in range(1, NIB):
                    c0, c1 = ic * IBLK, (ic + 1) * IBLK
                    for h in range(CH):
                        in_chunk(xr, xr_sb, h, c0, c1)
                        in_chunk(yr, yr_sb, h, c0, c1)
                for ic in range(NIB):
                    c0, c1 = ic * IBLK, (ic + 1) * IBLK
                    for w_sb, b_sb, src, dst in (
                        (wq_sb, bq_sb, xr_sb, q4),
                        (wk_sb, bk_sb, yr_sb, k4),
                    ):
                        ps = psA.tile([D, IBLK], F32, name=f"qk_{ic}", tag="qk_ps")
                        for h in range(CH):
                            nc.tensor.matmul(
                                ps[:],
                                w_sb[:, h * D:(h + 1) * D],
                                src[:, h * HW + c0: h * HW + c1],
                                start=(h == 0),
                                stop=(h == CH - 1),
                            )
                        nc.vector.tensor_scalar_add(
                            dst[0:D, c0:c1], ps[:], b_sb[:, 0:1]
                        )
                        for g in range(1, 4):
                            nc.gpsimd.dma_start(
                                dst[32 * g:32 * (g + 1), c0:c1], dst[0:D, c0:c1]
                            )
                    for jt in range(4 * ic, 4 * ic + 4):
                        ps = psA.tile([128, C], F32, name=f"vt_{jt}", tag="vt_ps")
                        for h in range(CH):
                            nc.tensor.matmul(
                                ps[:],
                                yr_sb[:, h * HW + jt * 128: h * HW + (jt + 1) * 128],
                                wv_sb[:, h * C:(h + 1) * C],
                                start=(h == 0),
                                stop=(h == CH - 1),
                            )
                        nc.vector.tensor_copy(vt[:, jt * C:(jt + 1) * C], ps[:])

            with (
                tc.tile_pool(name="ptp", bufs=2) as ptp,
                tc.tile_pool(name="wrk", bufs=2) as wrk,
                tc.tile_pool(name="psE", bufs=1, space="PSUM") as psE,
                tc.tile_pool(name="psAV", bufs=1, space="PSUM") as psAV,
            ):
                def make_tail(n, av, den):
                    def tail():
                        rgb = wrk.tile([128, IBLK], F32, name=f"rgb_{n}", tag="rgb")
                        nc.vector.reciprocal(rgb[:], den[:])
                        rgbg = wrk.tile([128, IBLK], F32, name=f"rgbg_{n}", tag="rgbg")
                        nc.vector.tensor_scalar(
                            rgbg[:], rgb[:], gm_sb[:, 0:1], None, MULT
                        )
                        for ch in range(CH):
                            xf_t = wrk.tile([128, IBLK], F32,
                                            name=f"xf_{n}_{ch}", tag="xf")
                            nc.sync.dma_start(
                                xf_t[:],
                                xf[ch * 128:(ch + 1) * 128, n * IBLK:(n + 1) * IBLK],
                            )
                            tmp = wrk.tile([128, IBLK], F32,
                                           name=f"tmp_{n}_{ch}", tag="tmp")
                            nc.vector.tensor_tensor(tmp[:], av[ch][:], rgbg[:], MULT)
                            ot = wrk.tile([128, IBLK], F32, name=f"ot_{n}_{ch}", tag="ot")
                            nc.vector.scalar_tensor_tensor(
                                ot[:], tmp[:], gbv_sb[:, ch:ch + 1], xf_t[:], ADD, ADD
                            )
                            nc.sync.dma_start(
                                out[ch * 128:(ch + 1) * 128, n * IBLK:(n + 1) * IBLK],
                                ot[:],
                            )
                    return tail

                ones_pair = ones_sb[:].rearrange("P (s c) -> P s c", s=2)

                pending_tail = None
                for n in range(NIB):
                    pt = ptp.tile([128, NJT * IBLK], FP8, name=f"pt_{n}", tag="pt")
                    av = [
                        psAV.tile([128, IBLK], F32, name=f"av{ch}_{n}", tag=f"av{ch}")
                        for ch in range(CH)
                    ]
                    den = psAV.tile([128, IBLK], F32, name=f"den_{n}", tag="den")

                    def av_pairs(g, pt=pt, av=av, den=den, n=n):
                        # DoubleRow AV + denominator for the 2 j-tile pairs of
                        # group g: virtual K=256 contracts two j-tiles at once
                        for p in (2 * g, 2 * g + 1):
                            ptp_ap = pt[:, 2 * p * IBLK:(2 * p + 2) * IBLK].rearrange(
                                "P (s N) -> P s N", s=2
                            )
                            vtp_ap = vt[:, 2 * p * C:(2 * p + 2) * C].rearrange(
                                "P (s c) -> P s c", s=2
                            )
                            for ch in range(CH):
                                nc.tensor.matmul(
                                    av[ch][:],
                                    vtp_ap[:, :, ch * 128:(ch + 1) * 128],
                                    ptp_ap,
                                    start=(p == 0),
                                    stop=(p == NPAIR - 1),
                                    perf_mode=DROW,
                                    skip_group_check=True,
                                )
                            nc.tensor.matmul(
                                den[:],
                                ones_pair,
                                ptp_ap,
                                start=(p == 0),
                                stop=(p == NPAIR - 1),
                                perf_mode=DROW,
                                skip_group_check=True,
                            )

                    for g in range(NG):
                        et_group(n, g, pt)
                        if g == 0 and pending_tail is not None:
                            pending_tail()
                            pending_tail = None
                        if g >= 2:
                            av_pairs(g - 2)
                    av_pairs(NG - 2)
                    av_pairs(NG - 1)
                    pending_tail = make_tail(n, av, den)
                pending_tail()
    nc.compile()
    return nc


_NC_CACHE = {}


def kernel(x, y, Wq, bq, Wk, bk, Wv, bv, gamma):
    assert x.shape == (B, C, 64, 64)
    xs = np.ascontiguousarray(x.reshape(B, C, HW).astype(np.float32))
    ys = np.ascontiguousarray(y.reshape(B, C, HW).astype(np.float32))
    wqT = np.ascontiguousarray(Wq.T.astype(np.float32))
    wkT = np.ascontiguousarray(Wk.T.astype(np.float32))
    wvT = np.ascontiguousarray(Wv.T.astype(np.float32))
    bqh = np.ascontiguousarray(bq.astype(np.float32).reshape(D, 1))
    bkh = np.ascontiguousarray(bk.astype(np.float32).reshape(D, 1))
    g = float(np.asarray(gamma).reshape(-1)[0])
    gbvh = np.ascontiguousarray((g * bv.astype(np.float32)).reshape(CH, 128).T)
    gmh = np.full((128, 1), g, dtype=np.float32)

    if "nc" not in _NC_CACHE:
        _NC_CACHE["nc"] = _build()
    nc = _NC_CACHE["nc"]

    in_maps = [
        {
            "xr": xs[b], "xf": xs[b], "yr": ys[b],
            "wqT": wqT, "wkT": wkT, "wvT": wvT,
            "bqd": bqh, "bkd": bkh, "gbvd": gbvh, "gmd": gmh,
        }
        for b in range(B)
    ]
    res = run_bass_kernel_spmd(nc, in_maps, list(range(B)))
    outs = np.stack([res.results[b]["out"] for b in range(B)])
    return outs.reshape(B, C, 64, 64).astype(np.float32)
